# revision 9
# baseline (speedup 1.0000x reference)
"""AccRNNCell Trainium2 kernel — 8-core data-parallel over batch.

Layout strategy: everything transposed ([feature, batch] on device) so matmul
outputs land directly in the layout the next matmul consumes — zero on-device
transposes. Weights are the stationary operand (lhsT = W as stored, [K, M]);
activations are the moving operand [K<=128, BL=64]. bf16 matmul inputs (enables
fast weight load), f32 PSUM accumulation, f32 running accumulator and outputs.

Host side: pre-transpose x to [F, T, BL] per core and cast to bf16; output
comes back [P, T, BL] f32 and is transposed to [BL, T, P].
"""

import numpy as np
import ml_dtypes

import concourse.bass as bass
from concourse import bacc
import concourse.mybir as mybir
import concourse.tile as tile
from concourse.bass import ds
from concourse.bass_utils import run_bass_kernel_spmd

L = 3
U = 512
P = 32
F = 64
B = 512
T = 512
NCORES = 8
BL = B // NCORES          # batch rows per core = 64
UK = U // 128             # 4 k/m chunks of 128
UNROLL = 8                # timesteps per For_i body

BF16 = mybir.dt.bfloat16
F32 = mybir.dt.float32
ADD = mybir.AluOpType.add


def build_graph(t_steps=T, unroll=UNROLL, debug=False):
    """Build the single-core Bass graph (same graph runs SPMD on 8 cores)."""
    assert t_steps % unroll == 0
    nblk = t_steps // unroll
    nc = bacc.Bacc()

    # ---- DRAM parameters (per-core shard views) ----
    x_d = nc.declare_dram_parameter("xT", [F, nblk, unroll, BL], BF16, isOutput=False)
    wa_d = nc.declare_dram_parameter("wa", [L, UK, 128, U], BF16, isOutput=False)
    wb0_d = nc.declare_dram_parameter("wb0", [F + P, U], BF16, isOutput=False)
    wbr_d = nc.declare_dram_parameter("wbr", [L - 1, UK, 128, U], BF16, isOutput=False)
    wc_d = nc.declare_dram_parameter("wc", [L, UK, 128, U], BF16, isOutput=False)
    wout_d = nc.declare_dram_parameter("wout", [UK, 128, P], BF16, isOutput=False)
    bs_d = nc.declare_dram_parameter("bs", [L, UK, 128, BL], F32, isOutput=False)   # bA + bB combined, broadcast over batch
    bc_d = nc.declare_dram_parameter("bc", [L, UK, 128, BL], F32, isOutput=False)
    bout_d = nc.declare_dram_parameter("bout", [P, BL], F32, isOutput=False)
    y_d = nc.declare_dram_parameter("yT", [P, nblk, unroll, BL], F32, isOutput=True)
    dbg_d = None
    if debug:
        dbg_d = nc.declare_dram_parameter("dbg", [L, 128, UK, BL], BF16, isOutput=True)

    with tile.TileContext(nc) as tc:
        with (
            tc.tile_pool(name="const", bufs=1) as cpool,
            tc.tile_pool(name="state", bufs=1) as stpool,
            tc.tile_pool(name="ct", bufs=3) as ctpool,
            tc.tile_pool(name="accbf", bufs=2) as abpool,
            tc.tile_pool(name="ystage", bufs=2) as ypool,
            tc.tile_pool(name="ps_s", bufs=4, space="PSUM") as spool,
            tc.tile_pool(name="ps_c", bufs=3, space="PSUM") as cpspool,
            tc.tile_pool(name="ps_r", bufs=1, space="PSUM") as rpool,
        ):
            # ---- load weights/biases/x into SBUF once ----
            wa_sb = [cpool.tile([128, UK, U], BF16, tag=f"wa{l}", name=f"wa{l}") for l in range(L)]
            wc_sb = [cpool.tile([128, UK, U], BF16, tag=f"wc{l}", name=f"wc{l}") for l in range(L)]
            wbr_sb = [cpool.tile([128, UK, U], BF16, tag=f"wbr{l}", name=f"wbr{l}") for l in range(L - 1)]
            wb0x_sb = cpool.tile([F, U], BF16, tag="wb0x")
            wb0a_sb = cpool.tile([P, U], BF16, tag="wb0a")
            wout_sb = cpool.tile([128, UK, P], BF16, tag="wout")
            bs_sb = cpool.tile([128, L, UK, BL], F32, tag="bs")
            bc_sb = cpool.tile([128, L, UK, BL], F32, tag="bc")
            bout_sb = cpool.tile([P, BL], F32, tag="bout")
            x_sb = cpool.tile([F, nblk, unroll, BL], BF16, tag="x")

            for l in range(L):
                for k in range(UK):
                    nc.sync.dma_start(out=wa_sb[l][:, k, :], in_=wa_d[l, k])
                    nc.sync.dma_start(out=wc_sb[l][:, k, :], in_=wc_d[l, k])
                    if l < L - 1:
                        nc.sync.dma_start(out=wbr_sb[l][:, k, :], in_=wbr_d[l, k])
                    nc.sync.dma_start(out=bs_sb[:, l, k, :], in_=bs_d[l, k])
                    nc.sync.dma_start(out=bc_sb[:, l, k, :], in_=bc_d[l, k])
            nc.sync.dma_start(out=wb0x_sb[:, :], in_=wb0_d[0:F, :])
            nc.sync.dma_start(out=wb0a_sb[:, :], in_=wb0_d[F:F + P, :])
            for k in range(UK):
                nc.sync.dma_start(out=wout_sb[:, k, :], in_=wout_d[k])
            nc.sync.dma_start(out=bout_sb[:, :], in_=bout_d[:, :])
            nc.sync.dma_start(out=x_sb[:, :, :, :], in_=x_d[:, :, :, :])

            # ---- persistent state ----
            sT = [stpool.tile([128, UK, BL], BF16, tag=f"sT{l}", name=f"sT{l}") for l in range(L)]
            accT = stpool.tile([P, BL], F32, tag="accT")
            for l in range(L):
                nc.vector.memset(sT[l][:, :, :], 0.0)
            nc.vector.memset(accT[:, :], 0.0)

            def step(ib, j, y_stage):
                """One timestep: t = ib*unroll + j.

                PSUM discipline: start=True clears the WHOLE bank, so each
                accumulation group (one m-chunk of one layer) owns a private
                bank-sized tile until its DVE read frees the slot.
                """
                # A-matmuls for all layers first — they only need last step's
                # states, so the PE can chew on them while the serial
                # B->C chain of this step trickles through DVE copies.
                ps_s = []
                for l in range(L):
                    ps_l = []
                    for m in range(UK):
                        ps = spool.tile([128, BL], F32, tag="ps_s", name="ps_s")
                        ps_l.append(ps)
                        for k in range(UK):
                            nc.tensor.matmul(
                                ps[:, :],
                                wa_sb[l][:, k, ds(m * 128, 128)],
                                sT[l][:, k, :],
                                start=(k == 0),
                                stop=False,
                            )
                    ps_s.append(ps_l)

                # bf16 copy of acc for the B0 matmul rhs
                acc_bf = abpool.tile([P, BL], BF16, tag="acc_bf")
                nc.vector.tensor_copy(out=acc_bf[:, :], in_=accT[:, :])

                prev_ct = None
                for l in range(L):
                    # B-path: continue accumulation in each m-chunk's bank
                    for m in range(UK):
                        ps = ps_s[l][m]
                        if l == 0:
                            nc.tensor.matmul(
                                ps[:, :],
                                wb0x_sb[:, ds(m * 128, 128)],
                                x_sb[:, ds(ib, 1), j, :],
                                start=False,
                                stop=False,
                            )
                            nc.tensor.matmul(
                                ps[:, :],
                                wb0a_sb[:, ds(m * 128, 128)],
                                acc_bf[:, :],
                                start=False,
                                stop=True,
                            )
                        else:
                            for k in range(UK):
                                nc.tensor.matmul(
                                    ps[:, :],
                                    wbr_sb[l - 1][:, k, ds(m * 128, 128)],
                                    prev_ct[:, k, :],
                                    start=False,
                                    stop=(k == UK - 1),
                                )
                    # s_new = psum + bias  (frees the s bank for the next layer)
                    for m in range(UK):
                        nc.vector.tensor_tensor(
                            out=sT[l][:, m, :],
                            in0=ps_s[l][m][:, :],
                            in1=bs_sb[:, l, m, :],
                            op=ADD,
                        )
                    # C-path: private bank per m-chunk
                    ct = ctpool.tile([128, UK, BL], BF16, tag="ct")
                    for m in range(UK):
                        ps_c = cpspool.tile([128, BL], F32, tag="ps_c", name="ps_c")
                        for k in range(UK):
                            nc.tensor.matmul(
                                ps_c[:, :],
                                wc_sb[l][:, k, ds(m * 128, 128)],
                                sT[l][:, k, :],
                                start=(k == 0),
                                stop=(k == UK - 1),
                            )
                        nc.vector.tensor_tensor(
                            out=ct[:, m, :],
                            in0=ps_c[:, :],
                            in1=bc_sb[:, l, m, :],
                            op=ADD,
                        )
                    prev_ct = ct

                # output head
                ps_r = rpool.tile([P, BL], F32, tag="ps_r")
                for k in range(UK):
                    nc.tensor.matmul(
                        ps_r[:, :],
                        wout_sb[:, k, :],
                        prev_ct[:, k, :],
                        start=(k == 0),
                        stop=(k == UK - 1),
                    )
                nc.vector.tensor_tensor(
                    out=y_stage[:, j, :],
                    in0=ps_r[:, :],
                    in1=bout_sb[:, :],
                    op=ADD,
                )
                nc.vector.tensor_tensor(
                    out=accT[:, :],
                    in0=accT[:, :],
                    in1=y_stage[:, j, :],
                    op=ADD,
                )

            def block_body(ib):
                y_stage = ypool.tile([P, unroll, BL], F32, tag="y_stage")
                for j in range(unroll):
                    step(ib, j, y_stage)
                nc.sync.dma_start(out=y_d[:, ds(ib, 1), :, :], in_=y_stage[:, :, :])

            if nblk == 1:
                block_body(0)
            else:
                with tc.For_i(0, nblk, 1) as ib:
                    block_body(ib)
            if debug:
                for l in range(L):
                    nc.sync.dma_start(out=dbg_d[l], in_=sT[l][:, :, :])

    nc.finalize()
    return nc


def _prep_inputs(x, WA, bA, WB0, bB0, WBr, bBr, WC, bC, Wout, bout, t_steps=T, unroll=UNROLL):
    """Host-side shard + transpose + cast. Returns in_maps for 8 cores."""
    bf = ml_dtypes.bfloat16
    nblk = t_steps // unroll
    wa = np.ascontiguousarray(WA.reshape(L, UK, 128, U)).astype(bf)
    wb0 = np.ascontiguousarray(WB0).astype(bf)
    wbr = np.ascontiguousarray(WBr.reshape(L - 1, UK, 128, U)).astype(bf)
    wc = np.ascontiguousarray(WC.reshape(L, UK, 128, U)).astype(bf)
    wout = np.ascontiguousarray(Wout.reshape(UK, 128, P)).astype(bf)
    bs = bA + np.concatenate([bB0[None], bBr], axis=0)  # [L, U]
    bs = np.broadcast_to(bs.reshape(L, UK, 128, 1), (L, UK, 128, BL))
    bs = np.ascontiguousarray(bs).astype(np.float32)
    bc = np.broadcast_to(bC.reshape(L, UK, 128, 1), (L, UK, 128, BL))
    bc = np.ascontiguousarray(bc).astype(np.float32)
    bout = np.broadcast_to(np.asarray(bout).reshape(P, 1), (P, BL))
    bout = np.ascontiguousarray(bout).astype(np.float32)

    in_maps = []
    for c in range(NCORES):
        xs = x[c * BL:(c + 1) * BL, :t_steps, :]          # [BL, t, F]
        xT = np.ascontiguousarray(xs.transpose(2, 1, 0))  # [F, t, BL]
        xT = xT.reshape(F, nblk, unroll, BL).astype(bf)
        in_maps.append({
            "xT": xT, "wa": wa, "wb0": wb0, "wbr": wbr, "wc": wc,
            "wout": wout, "bs": bs, "bc": bc, "bout": bout,
        })
    return in_maps


def _gather_output(results, t_steps=T):
    """results[i]['yT'] [P, nblk, unroll, BL] -> full y [B, t, P] f32."""
    outs = []
    for c in range(NCORES):
        yT = np.asarray(results[c]["yT"], dtype=np.float32).reshape(P, t_steps, BL)
        outs.append(np.ascontiguousarray(yT.transpose(2, 1, 0)))  # [BL, t, P]
    return np.concatenate(outs, axis=0)


def kernel(x, WA, bA, WB0, bB0, WBr, bBr, WC, bC, Wout, bout):
    nc = build_graph(T, UNROLL)
    in_maps = _prep_inputs(x, WA, bA, WB0, bB0, WBr, bBr, WC, bC, Wout, bout)
    res = run_bass_kernel_spmd(nc, in_maps, core_ids=list(range(NCORES)))
    return _gather_output(res.results)
